# revision 1
# baseline (speedup 1.0000x reference)
"""Trainium2 Bass kernel for a CapsuleNet dynamic-routing layer.

Math (per batch element b):
    u_hat[b,i,o,d] = sum_k W[i,o,d,k] * x[b,i,k]      # B=256, IC=1152, OC=10, OD=16, ID=8
    b_log = 0
    for it in 0..2:
        c = softmax(b_log, axis=o)
        s[b,o,d] = sum_i c[b,i,o] * u_hat[b,i,o,d]
        v = squash(s)
        if it < 2: b_log += sum_d u_hat * v

Sharding: data-parallel over B across 8 cores (32 local rows), W replicated.

Per-core layout: partition axis p = bp*16 + ii (bp = b%8, ii = i%16); the
u_hat build contracts q = ii*8 + k with a host-prepacked block-diagonal x
(lhsT) against the W stack (rhs), one matmul per (iblk, bblk).  u_hat lives in
SBUF as [128(p), 72(iblk), 4(bblk), 160(o*16+d)] fp16.  The c-weighted
i-reduction runs on the PE via a block-diagonal c per bblk accumulating into
PSUM over all iblk, then a 0/1 o-diagonal mask multiply and a tiny second
matmul extract s.  The agreement step (sum_d u*v) is a fp16 DVE multiply
(2x mode) + in-place pairwise-add tree.  fp16 everywhere keeps rounding ~5e-4.
"""

import sys

sys.path.insert(0, "/opt/trn_rl_repo")

from contextlib import ExitStack

import numpy as np

import concourse.bass as bass
import concourse.tile as tile
from concourse import mybir

# fp16 (not bf16): same PE/DVE throughput for 2-byte dtypes, 4 extra mantissa
# bits; all tensors here have magnitudes well inside fp16 normal range.
BF = mybir.dt.float16
F32 = mybir.dt.float32
AX = mybir.AxisListType
AF = mybir.ActivationFunctionType

N_CORES = 8
B_FULL, IC, OC, OD, ID = 256, 1152, 10, 16, 8
B_LOC = B_FULL // N_CORES          # 32
NIB = IC // 16                     # 72 i-blocks of 16
NBB = B_LOC // 8                   # 4 b-blocks of 8
F = OC * OD                        # 160


def _squash(nc, smp, ps, scale, vout):
    """vout = squash(scale * ps) with ps an [8, 160] psum slab (f32).

    squash(s) = (n^2/(1+n^2)) * s/(n + 1e-8),  n = ||s||_2 over d.
    """
    P = 8
    sq = smp.tile([P, F], F32, tag="sq")
    nc.scalar.activation(sq[:], ps[:], AF.Square, scale=float(scale))
    n2 = smp.tile([P, OC], F32, tag="n2")
    nc.vector.tensor_reduce(
        n2[:], sq[:].rearrange("p (o d) -> p o d", d=OD), axis=AX.X,
        op=mybir.AluOpType.add)
    n1 = smp.tile([P, OC], F32, tag="n1")
    nc.scalar.add(n1[:], n2[:], 1.0)
    r1 = smp.tile([P, OC], F32, tag="r1")
    nc.vector.reciprocal(r1[:], n1[:])
    sn = smp.tile([P, OC], F32, tag="sn")
    nc.scalar.sqrt(sn[:], n2[:])
    sne = smp.tile([P, OC], F32, tag="sne")
    nc.vector.tensor_scalar_add(sne[:], sn[:], 1e-8)
    r2 = smp.tile([P, OC], F32, tag="r2")
    nc.vector.reciprocal(r2[:], sne[:])
    f1 = smp.tile([P, OC], F32, tag="f1")
    nc.vector.tensor_mul(f1[:], n2[:], r1[:])
    nc.vector.tensor_mul(f1[:], f1[:], r2[:])
    if scale != 1.0:
        nc.scalar.mul(f1[:], f1[:], float(scale))
    nc.vector.tensor_mul(
        vout[:].rearrange("p (o d) -> p o d", d=OD),
        ps[:].rearrange("p (o d) -> p o d", d=OD),
        f1[:].unsqueeze(-1).broadcast_to((P, OC, OD)))


def _split_multiwait(nc):
    """The walrus in this container encodes at most ONE semaphore wait on
    Matmult/Ldweights and HWDGE DMACopy instructions ("Too many sync wait
    commands").  Hoist excess waits onto same-engine NoOps placed directly
    before the instruction - position-identical semantics, ~2 cycles each.
    SWDGE (Pool-queue) DMAs handle multi-waits fine and are left alone.
    """
    for fn in nc.m.functions:
        for bb in fn.blocks:
            out = []
            k = 0
            for ins in bb.instructions:
                si = ins.sync_info
                waits = list(si.on_wait) if si is not None and si.on_wait else []
                limit = 1
                if ins.opcode == "DMACopy":
                    q = str(getattr(ins, "queue", "") or "")
                    if "HW" in q and len(waits) > 1:
                        # HWDGE queue instructions can't be gated by engine
                        # NoOps; the kernel must be structured to avoid this.
                        raise AssertionError(
                            f"HWDGE DMA {ins.name} has {len(waits)} waits: {ins}")
                if len(waits) > limit:
                    for w in waits[:-limit]:
                        nop = mybir.InstNoOp(name=f"{ins.name}-wn{k}", ins=[], outs=[])
                        k += 1
                        nop.engine = ins.engine
                        nop.sync_info = mybir.SyncInfo(on_wait=[w], on_update=[])
                        out.append(nop)
                    ins.sync_info = mybir.SyncInfo(
                        on_wait=waits[-limit:],
                        on_update=list(si.on_update) if si.on_update else [])
                out.append(ins)
            bb.instructions = out


def build_program(split_waits=True):
    """split_waits=True applies the walrus 1-wait workaround (required for
    hardware compiles); CoreSim/TimelineSim need the unsplit program."""
    nc = bass.Bass()
    bd_d = nc.declare_dram_parameter("bd", [8, 128, 9, NBB, 128], BF, isOutput=False)
    xt_d = nc.declare_dram_parameter("xt", [128, NIB, B_LOC], BF, isOutput=False)
    ws_d = nc.declare_dram_parameter("ws", [8, 128, 9, F], BF, isOutput=False)
    msk_d = nc.declare_dram_parameter("msk", [80, F], BF, isOutput=False)
    o128_d = nc.declare_dram_parameter("o128", [128, 8], BF, isOutput=False)
    o80_d = nc.declare_dram_parameter("o80", [80, 8], BF, isOutput=False)
    sel_d = nc.declare_dram_parameter("sel", [8, 128], BF, isOutput=False)
    out_d = nc.declare_dram_parameter("out", [B_LOC, F], F32, isOutput=True)

    with ExitStack() as ctx:
        tc = ctx.enter_context(tile.TileContext(nc))
        st = ctx.enter_context(tc.tile_pool(name="st", bufs=1))
        bdp = ctx.enter_context(tc.tile_pool(name="bdp", bufs=2))
        wsp = ctx.enter_context(tc.tile_pool(name="wsp", bufs=2))
        wsp0 = ctx.enter_context(tc.tile_pool(name="wsp0", bufs=2))
        y2p = ctx.enter_context(tc.tile_pool(name="y2p", bufs=2))
        t1p = ctx.enter_context(tc.tile_pool(name="t1p", bufs=2))
        tsp = ctx.enter_context(tc.tile_pool(name="tsp", bufs=2))
        mkp = ctx.enter_context(tc.tile_pool(name="mkp", bufs=2))
        vxp = ctx.enter_context(tc.tile_pool(name="vxp", bufs=2))
        smp = ctx.enter_context(tc.tile_pool(name="smp", bufs=4))
        pbig = ctx.enter_context(tc.tile_pool(name="pbig", bufs=4, space="PSUM"))
        psml = ctx.enter_context(tc.tile_pool(name="psml", bufs=4, space="PSUM"))

        # --- persistent tiles ---
        u_hat = st.tile([128, NIB, NBB, F], BF, tag="u_hat")
        blg = st.tile([128, NBB, OC, NIB], BF, tag="blg")
        c_sb = st.tile([128, NBB, OC, NIB], BF, tag="c_sb")
        cb0 = st.tile([128, 80, NIB], BF, tag="cb0")
        cb1 = st.tile([128, 80, NIB], BF, tag="cb1")
        msk_sb = st.tile([80, F], BF, tag="msk_sb")
        o128_sb = st.tile([128, 8], BF, tag="o128_sb")
        o80_sb = st.tile([80, 8], BF, tag="o80_sb")
        sel_sb = st.tile([8, 128], BF, tag="sel_sb")
        xt_sb = st.tile([128, NIB, B_LOC], BF, tag="xt_sb")
        v8 = [st.tile([8, F], BF, tag=f"v8_{i}", name=f"v8_{i}") for i in range(NBB)]
        of8 = [st.tile([8, F], F32, tag=f"of8_{i}", name=f"of8_{i}") for i in range(NBB)]

        # --- input loads + zero-init of the c-blockdiag ---
        nc.sync.dma_start(out=msk_sb[:], in_=msk_d[:])
        nc.sync.dma_start(out=o128_sb[:], in_=o128_d[:])
        nc.sync.dma_start(out=o80_sb[:], in_=o80_d[:])
        nc.sync.dma_start(out=sel_sb[:], in_=sel_d[:])
        nc.scalar.dma_start(out=xt_sb[:], in_=xt_d[:])
        nc.gpsimd.memset(cb0[:], 0.0)
        nc.gpsimd.memset(cb1[:], 0.0)

        # --- pass 1: iter-0 uniform-c reduction s0 = 0.1*sum_i u_hat computed
        # directly as x @ W over the full (i,k) contraction from xt/ws -- 72
        # matmuls into one [32,160] psum.  Runs before the build so iter-0's
        # squash/agreement overlap the u_hat build below. ---
        ps0 = [psml.tile([8, F], F32, tag="psml", name=f"ps0_{i}")
               for i in range(NBB)]
        for e in range(8):
            wst0 = wsp0.tile([128, 9, F], BF, tag="wst0")
            nc.gpsimd.dma_start(out=wst0[:], in_=ws_d[e])
            for j in range(9):
                iblk = e * 9 + j
                for bblk in range(NBB):
                    nc.tensor.matmul(
                        ps0[bblk][:],
                        lhsT=xt_sb[:, iblk, bblk * 8:(bblk + 1) * 8],
                        rhs=wst0[:, j, :],
                        start=(iblk == 0), stop=(iblk == NIB - 1))

        # --- pass 2: build u_hat, one matmul per (iblk, bblk), K=128=(ii,k),
        # N=160; PSUM drained to SBUF split evenly across ACT and DVE ---
        for e in range(8):
            bdt = bdp.tile([128, 9, NBB, 128], BF, tag="bdt")
            nc.gpsimd.dma_start(out=bdt[:], in_=bd_d[e])
            wst = wsp.tile([128, 9, F], BF, tag="wst")
            nc.gpsimd.dma_start(out=wst[:], in_=ws_d[e])
            for j in range(9):
                iblk = e * 9 + j
                for h in range(2):
                    ps = pbig.tile([128, 2, F], F32, tag="pbig")
                    for bb in range(2):
                        nc.tensor.matmul(
                            ps[:, bb, :], lhsT=bdt[:, j, h * 2 + bb, :],
                            rhs=wst[:, j, :], start=True, stop=True)
                    if (2 * iblk + h) % 2 == 0:
                        nc.scalar.copy(u_hat[:, iblk, h * 2:(h + 1) * 2, :], ps[:])
                    else:
                        nc.vector.tensor_copy(
                            u_hat[:, iblk, h * 2:(h + 1) * 2, :], ps[:])

        def agreement_bblk(bblk, first):
            pvx = psml.tile([128, F], F32, tag="psml", name=f"pvx{bblk}")
            nc.tensor.matmul(
                pvx[:], lhsT=sel_sb[:], rhs=v8[bblk][:], start=True, stop=True)
            vx = vxp.tile([128, F], BF, tag="vx", name=f"vx{bblk}")
            nc.scalar.copy(vx[:], pvx[:])
            if True:
                for q in range(4):
                    # q==3 runs on the otherwise-idle GPSIMD (slower per-op,
                    # but it frees the DVE, the agreement bottleneck)
                    eng = nc.gpsimd if q == 3 else nc.vector
                    sl = slice(q * 18, (q + 1) * 18)
                    y2 = y2p.tile([128, 18, F], BF, tag="y2")
                    eng.tensor_mul(
                        y2[:], u_hat[:, sl, bblk, :],
                        vx[:].unsqueeze(1).broadcast_to((128, 18, F)))
                    # in-place pairwise-add tree over d (fp16, 2x mode)
                    y2v = y2[:].rearrange("p j (o d) -> p j o d", d=OD)
                    eng.tensor_add(
                        y2v[:, :, :, 0:8], y2v[:, :, :, 0:8], y2v[:, :, :, 8:16])
                    eng.tensor_add(
                        y2v[:, :, :, 0:4], y2v[:, :, :, 0:4], y2v[:, :, :, 4:8])
                    eng.tensor_add(
                        y2v[:, :, :, 0:2], y2v[:, :, :, 0:2], y2v[:, :, :, 2:4])
                    dst = blg[:, bblk, :, sl].transpose([0, 2, 1])
                    if first:
                        eng.tensor_add(
                            dst, y2v[:, :, :, 0], y2v[:, :, :, 1])
                    else:
                        ts = tsp.tile([128, 18, OC], BF, tag="ts")
                        eng.tensor_add(ts[:], y2v[:, :, :, 0], y2v[:, :, :, 1])
                        eng.tensor_add(dst, dst, ts[:])

        # --- iter 0: c uniform -> s = 0.1 * sum_i u_hat (accumulated above) ---
        for bblk in range(NBB):
            _squash(nc, smp, ps0[bblk][:], 0.1, v8[bblk])
            agreement_bblk(bblk, first=True)

        # --- iters 1, 2: fully per-bblk pipelines so PE stage-1 of one
        # b-block overlaps the DVE agreement/softmax of another ---
        for it in (1, 2):
            for bblk in range(NBB):
                # softmax over o for this b-block (no max-sub: |logits| << 1)
                nc.scalar.activation(
                    c_sb[:, bblk, :, :], blg[:, bblk, :, :], AF.Exp)
                cf = c_sb[:, bblk, :, :].transpose([0, 2, 1])   # [p, i, o]
                sm = smp.tile([128, NIB], F32, tag="sm")
                nc.vector.tensor_reduce(
                    sm[:], cf, axis=AX.X, op=mybir.AluOpType.add)
                rr = smp.tile([128, NIB], F32, tag="rr")
                nc.vector.reciprocal(rr[:], sm[:])
                nc.vector.tensor_mul(
                    cf, cf, rr[:].unsqueeze(-1).broadcast_to((128, NIB, OC)))

                # stage 1+2: s = diag(C^T U) via blockdiag-c, o-mask, reduce
                cbt = (cb0, cb1)[bblk % 2]
                for bp in range(8):
                    nc.gpsimd.dma_start(
                        out=cbt[bp * 16:(bp + 1) * 16, bp * 10:(bp + 1) * 10, :],
                        in_=c_sb[bp * 16:(bp + 1) * 16, bblk, :, :])
                ps1 = pbig.tile([80, F], F32, tag="pbig", name=f"ps1_{bblk}")
                for j in range(NIB):
                    nc.tensor.matmul(
                        ps1[:], lhsT=cbt[:, :, j],
                        rhs=u_hat[:, j, bblk, :],
                        start=(j == 0), stop=(j == NIB - 1))
                mskd = mkp.tile([80, F], BF, tag="mskd")
                nc.vector.tensor_mul(mskd[:], ps1[:], msk_sb[:])
                psv = psml.tile([8, F], F32, tag="psml")
                nc.tensor.matmul(
                    psv[:], lhsT=o80_sb[:], rhs=mskd[:], start=True, stop=True)
                if it == 1:
                    _squash(nc, smp, psv, 1.0, v8[bblk])
                    agreement_bblk(bblk, first=False)
                else:
                    _squash(nc, smp, psv, 1.0, of8[bblk])
                    nc.gpsimd.dma_start(
                        out=out_d[bblk * 8:(bblk + 1) * 8, :], in_=of8[bblk][:])

    if split_waits:
        _split_multiwait(nc)
    return nc


def _host_inputs(x, W):
    """Per-core input maps from full x [256,1152,8] f32, W [1,1152,10,16,8] f32."""
    bf = np.float16
    W0 = np.asarray(W[0], dtype=np.float32)
    # ws[e, q=(ii,k), j, (o,d)] = W[(e*9+j)*16+ii, o, d, k]
    ws = np.ascontiguousarray(
        W0.reshape(8, 9, 16, OC, OD, ID).transpose(0, 2, 5, 1, 3, 4)
        .reshape(8, 128, 9, F)).astype(bf)
    msk = np.zeros((80, F), dtype=bf)
    for bpp in range(8):
        for o in range(OC):
            msk[bpp * 10 + o, o * OD:(o + 1) * OD] = 1.0
    o128 = np.zeros((128, 8), dtype=bf)
    for p in range(128):
        o128[p, p // 16] = 1.0
    o80 = np.zeros((80, 8), dtype=bf)
    for p in range(80):
        o80[p, p // 10] = 1.0
    sel = np.ascontiguousarray(o128.astype(np.float32).T).astype(bf)

    in_maps = []
    for c in range(N_CORES):
        xc = np.asarray(x[c * B_LOC:(c + 1) * B_LOC], dtype=np.float32)
        # bd[e, q=(ii,k), j, bb, m=(bp,ii')] = x[bb*8+bp, (e*9+j)*16+ii, k] iff ii'==ii
        r = xc.reshape(NBB, 8, 8, 9, 16, ID)          # [bb, bp, e, j, ii, k]
        bd6 = np.zeros((8, 16, ID, 9, NBB, 8, 16), dtype=np.float32)
        for ii in range(16):
            # [bb, bp, e, j, k] -> [e, k, j, bb, bp]
            bd6[:, ii, :, :, :, :, ii] = r[:, :, :, :, ii, :].transpose(2, 4, 3, 0, 1)
        bd = np.ascontiguousarray(bd6.reshape(8, 128, 9, NBB, 128)).astype(bf)
        # xt[q=(ii,k), iblk, b] = x[b, iblk*16+ii, k]
        xt = np.ascontiguousarray(
            xc.reshape(B_LOC, NIB, 16, ID).transpose(2, 3, 1, 0)
            .reshape(128, NIB, B_LOC)).astype(bf)
        in_maps.append(
            {"bd": bd, "xt": xt, "ws": ws, "msk": msk, "o128": o128, "o80": o80,
             "sel": sel})
    return in_maps


_NC_CACHE = {}


def kernel(x, W):
    from concourse.bass_utils import run_bass_kernel_spmd

    if "nc" not in _NC_CACHE:
        _NC_CACHE["nc"] = build_program()
    nc = _NC_CACHE["nc"]
    in_maps = _host_inputs(x, W)
    res = run_bass_kernel_spmd(nc, in_maps, core_ids=list(range(N_CORES)))
    out = np.concatenate([r["out"] for r in res.results], axis=0)
    return out.reshape(B_FULL, OC, OD).astype(np.float32)


if __name__ == "__main__":
    nc = build_program()
    print("program built ok")



# revision 11
# speedup vs baseline: 1.0739x; 1.0739x over previous
"""Trainium2 Bass kernel for a CapsuleNet dynamic-routing layer.

Math (per batch element b):
    u_hat[b,i,o,d] = sum_k W[i,o,d,k] * x[b,i,k]      # B=256, IC=1152, OC=10, OD=16, ID=8
    b_log = 0
    for it in 0..2:
        c = softmax(b_log, axis=o)
        s[b,o,d] = sum_i c[b,i,o] * u_hat[b,i,o,d]
        v = squash(s)
        if it < 2: b_log += sum_d u_hat * v

Sharding: data-parallel over B across 8 cores (32 local rows), W replicated.

Per-core layout: partition axis p = bp*16 + ii (bp = b%8, ii = i%16); the
u_hat build contracts q = ii*8 + k with a host-prepacked block-diagonal x
(lhsT) against the W stack (rhs), one matmul per (iblk, bblk).  u_hat lives in
SBUF as [128(p), 72(iblk), 4(bblk), 160(o*16+d)] fp16.  The c-weighted
i-reduction runs on the PE via a block-diagonal c per bblk accumulating into
PSUM over all iblk, then a 0/1 o-diagonal mask multiply and a tiny second
matmul extract s.  The agreement step (sum_d u*v) is a fp16 DVE multiply
(2x mode) + in-place pairwise-add tree, split DVE-heavy (gpsimd TT measured
~4x slower).  Big loads ride the two HWDGE queues (SP + ACT) so the gpsimd
ring only carries the c-scatter; PSUM drains go mostly to the otherwise-idle
ACT engine.  fp16 everywhere keeps rounding ~5e-4.
"""

import sys

sys.path.insert(0, "/opt/trn_rl_repo")

from contextlib import ExitStack

import numpy as np

import concourse.bass as bass
import concourse.tile as tile
from concourse import mybir

# fp16 (not bf16): same PE/DVE throughput for 2-byte dtypes, 4 extra mantissa
# bits; all tensors here have magnitudes well inside fp16 normal range.
BF = mybir.dt.float16
F32 = mybir.dt.float32
AX = mybir.AxisListType
AF = mybir.ActivationFunctionType
ALU = mybir.AluOpType

N_CORES = 8
B_FULL, IC, OC, OD, ID = 256, 1152, 10, 16, 8
B_LOC = B_FULL // N_CORES          # 32
NIB = IC // 16                     # 72 i-blocks of 16
NBB = B_LOC // 8                   # 4 b-blocks of 8
F = OC * OD                        # 160

# Agreement iblk split: 3 DVE slices + 1 gpsimd slice (gpsimd TT ~4x slower).
AGR_SLICES = ((0, 20, "v"), (20, 40, "v"), (40, 60, "v"), (60, 72, "g"))
# Of the 144 [128,2,F] psum drains, 1-in-6 goes to DVE, rest to ACT.
DRAIN_DVE_EVERY = 6


def _squash(nc, smp, ps, scale, vout):
    """vout = squash(scale * ps) with ps an [P, 160] psum slab (f32).

    squash(s) = (n^2/(1+n^2)) * s/(n + 1e-8),  n = ||s||_2 over d.
    With n >= ~1e-2 here the 1e-8 is negligible: f = n/(1+n^2).
    """
    P = ps.shape[0]
    sq = smp.tile([P, F], F32, tag="sq")
    nc.scalar.activation(sq[:], ps[:], AF.Square, scale=float(scale))
    n2 = smp.tile([P, OC], F32, tag="n2")
    nc.vector.tensor_reduce(
        n2[:], sq[:].rearrange("p (o d) -> p o d", d=OD), axis=AX.X,
        op=ALU.add)
    n1 = smp.tile([P, OC], F32, tag="n1")
    nc.scalar.add(n1[:], n2[:], 1.0)
    r1 = smp.tile([P, OC], F32, tag="r1")
    nc.vector.reciprocal(r1[:], n1[:])
    sn = smp.tile([P, OC], F32, tag="sn")
    nc.scalar.sqrt(sn[:], n2[:])
    f1 = smp.tile([P, OC], F32, tag="f1")
    nc.vector.tensor_mul(f1[:], sn[:], r1[:])
    if scale != 1.0:
        nc.scalar.mul(f1[:], f1[:], float(scale))
    nc.vector.tensor_mul(
        vout[:].rearrange("p (o d) -> p o d", d=OD),
        ps[:].rearrange("p (o d) -> p o d", d=OD),
        f1[:].unsqueeze(-1).broadcast_to((P, OC, OD)))


def _split_multiwait(nc):
    """The walrus in this container encodes at most ONE semaphore wait on
    Matmult/Ldweights and HWDGE DMACopy instructions ("Too many sync wait
    commands").  Hoist excess waits onto same-engine NoOps placed directly
    before the instruction - position-identical semantics, ~2 cycles each.
    SWDGE (Pool-queue) DMAs handle multi-waits fine and are left alone.
    """
    for fn in nc.m.functions:
        for bb in fn.blocks:
            out = []
            k = 0
            for ins in bb.instructions:
                si = ins.sync_info
                waits = list(si.on_wait) if si is not None and si.on_wait else []
                limit = 1
                if ins.opcode == "DMACopy":
                    q = str(getattr(ins, "queue", "") or "")
                    if "HW" in q and len(waits) > 1:
                        # HWDGE descriptors encode one wait and can't be gated
                        # by engine NoOps.  Tile's redundant-wait optimizer is
                        # disabled (inc-6505), so these DMAs carry DMAHW-queue
                        # sems alongside the real engine-sem dep.  Here every
                        # such DMAHW wait is either dominated by the kept
                        # engine wait (the engine's last read of the reused
                        # buffer transitively waited the old DMA's queue sems)
                        # or orders against a disjoint-memory DMA; drop them.
                        eng_w = [w for w in waits if "DMAHW" not in w.ant_name]
                        if len(eng_w) != 1:
                            raise AssertionError(
                                f"HWDGE DMA {ins.name}: can't reduce waits "
                                f"{[w.ant_name for w in waits]} to one")
                        ins.sync_info = mybir.SyncInfo(
                            on_wait=eng_w,
                            on_update=list(si.on_update) if si.on_update else [])
                        out.append(ins)
                        continue
                if len(waits) > limit:
                    for w in waits[:-limit]:
                        nop = mybir.InstNoOp(name=f"{ins.name}-wn{k}", ins=[], outs=[])
                        k += 1
                        nop.engine = ins.engine
                        nop.sync_info = mybir.SyncInfo(on_wait=[w], on_update=[])
                        out.append(nop)
                    ins.sync_info = mybir.SyncInfo(
                        on_wait=waits[-limit:],
                        on_update=list(si.on_update) if si.on_update else [])
                out.append(ins)
            bb.instructions = out


def build_program(split_waits=True):
    """split_waits=True applies the walrus 1-wait workaround (required for
    hardware compiles); CoreSim/TimelineSim need the unsplit program."""
    nc = bass.Bass()
    bd_d = nc.declare_dram_parameter("bd", [8, 128, 9, NBB, 128], BF, isOutput=False)
    xt_d = nc.declare_dram_parameter("xt", [128, NIB, B_LOC], BF, isOutput=False)
    ws_d = nc.declare_dram_parameter("ws", [8, 128, 9, F], BF, isOutput=False)
    msk_d = nc.declare_dram_parameter("msk", [80, F], BF, isOutput=False)
    o80_d = nc.declare_dram_parameter("o80", [80, 8], BF, isOutput=False)
    sel_d = nc.declare_dram_parameter("sel", [32, NBB, 128], BF, isOutput=False)
    out_d = nc.declare_dram_parameter("out", [B_LOC, F], F32, isOutput=True)

    with ExitStack() as ctx:
        tc = ctx.enter_context(tile.TileContext(nc))
        st = ctx.enter_context(tc.tile_pool(name="st", bufs=1))
        bdp = ctx.enter_context(tc.tile_pool(name="bdp", bufs=2))
        y2p = ctx.enter_context(tc.tile_pool(name="y2p", bufs=2))
        t1p = ctx.enter_context(tc.tile_pool(name="t1p", bufs=2))
        tsp = ctx.enter_context(tc.tile_pool(name="tsp", bufs=2))
        mkp = ctx.enter_context(tc.tile_pool(name="mkp", bufs=2))
        vxp = ctx.enter_context(tc.tile_pool(name="vxp", bufs=2))
        smp = ctx.enter_context(tc.tile_pool(name="smp", bufs=4))
        pbig = ctx.enter_context(tc.tile_pool(name="pbig", bufs=4, space="PSUM"))
        psml = ctx.enter_context(tc.tile_pool(name="psml", bufs=4, space="PSUM"))

        # --- persistent tiles ---
        u_hat = st.tile([128, NIB, NBB, F], BF, tag="u_hat")
        ws_sb = st.tile([128, 8, 9, F], BF, tag="ws_sb")
        blg = st.tile([128, NBB, OC, NIB], BF, tag="blg")
        c_sb = st.tile([128, NBB, OC, NIB], BF, tag="c_sb")
        cb0 = st.tile([128, 80, NIB], BF, tag="cb0")
        cb1 = st.tile([128, 80, NIB], BF, tag="cb1")
        msk_sb = st.tile([80, F], BF, tag="msk_sb")
        o80_sb = st.tile([80, 8], BF, tag="o80_sb")
        sel_sb = st.tile([32, NBB, 128], BF, tag="sel_sb")
        xt_sb = st.tile([128, NIB, B_LOC], BF, tag="xt_sb")
        v32 = st.tile([32, F], BF, tag="v32")
        v8 = [st.tile([8, F], BF, tag=f"v8_{i}", name=f"v8_{i}") for i in range(NBB)]
        of8 = [st.tile([8, F], F32, tag=f"of8_{i}", name=f"of8_{i}") for i in range(NBB)]

        # --- input loads (both HWDGE queues) + zero-init of the c-blockdiag ---
        nc.sync.dma_start(out=msk_sb[:], in_=msk_d[:])
        nc.sync.dma_start(out=o80_sb[:], in_=o80_d[:])
        nc.sync.dma_start(out=sel_sb[:], in_=sel_d[:])
        nc.scalar.dma_start(out=xt_sb[:], in_=xt_d[:])
        # ws: [8(e), 128(q), 9, F] -> [128, 8, 9, F], halves on separate queues
        nc.scalar.dma_start(
            out=ws_sb[:, 0:4], in_=xw_view(ws_d, 0, 4))
        nc.sync.dma_start(
            out=ws_sb[:, 4:8], in_=xw_view(ws_d, 4, 8))
        nc.vector.memset(cb0[:], 0.0)
        nc.vector.memset(cb1[:], 0.0)

        # --- pass 1: iter-0 uniform-c reduction s0 = 0.1*sum_i u_hat computed
        # directly as x @ W over the full (i,k) contraction from xt/ws -- 72
        # m=32 matmuls into one [32,160] psum.  Runs before the build so
        # iter-0's squash/agreement overlap the u_hat build below. ---
        ps0 = psml.tile([32, F], F32, tag="psml", name="ps0")
        for e in range(8):
            for j in range(9):
                iblk = e * 9 + j
                nc.tensor.matmul(
                    ps0[:], lhsT=xt_sb[:, iblk, :], rhs=ws_sb[:, e, j, :],
                    start=(iblk == 0), stop=(iblk == NIB - 1))

        # --- pass 2: build u_hat, one matmul per (iblk, bblk), K=128=(ii,k),
        # N=160; PSUM drained to SBUF mostly on ACT (idle during build) ---
        for e in range(8):
            bdt = bdp.tile([128, 9, NBB, 128], BF, tag="bdt")
            nc.sync.dma_start(out=bdt[:], in_=bd_d[e])
            for j in range(9):
                for h in range(2):
                    ps = pbig.tile([128, 2, F], F32, tag="pbig")
                    for bb in range(2):
                        nc.tensor.matmul(
                            ps[:, bb, :], lhsT=bdt[:, j, h * 2 + bb, :],
                            rhs=ws_sb[:, e, j, :], start=True, stop=True)
                    idx = e * 18 + j * 2 + h
                    if idx % DRAIN_DVE_EVERY == 0:
                        nc.vector.tensor_copy(
                            u_hat[:, e * 9 + j, h * 2:(h + 1) * 2, :], ps[:])
                    else:
                        nc.scalar.copy(u_hat[:, e * 9 + j, h * 2:(h + 1) * 2, :], ps[:])

        def agreement_bblk(bblk, first, vsrc):
            # vsrc: (tile, nrows) - v32 [32,F] for iter 0, v8[bblk] [8,F] later.
            vt, nr = vsrc
            pvx = psml.tile([128, F], F32, tag="psml", name=f"pvx{bblk}")
            nc.tensor.matmul(
                pvx[:], lhsT=sel_sb[0:nr, bblk if nr == 32 else 0, :],
                rhs=vt[0:nr, :], start=True, stop=True)
            vx = vxp.tile([128, F], BF, tag="vx", name=f"vx{bblk}")
            nc.scalar.copy(vx[:], pvx[:])
            for (lo, hi, eng_key) in AGR_SLICES:
                eng = nc.vector if eng_key == "v" else nc.gpsimd
                n = hi - lo
                sl = slice(lo, hi)
                y2 = y2p.tile([128, 20, F], BF, tag="y2")
                eng.tensor_mul(
                    y2[:, :n, :], u_hat[:, sl, bblk, :],
                    vx[:].unsqueeze(1).broadcast_to((128, n, F)))
                # in-place pairwise-add tree over d (fp16, 2x mode)
                y2v = y2[:, :n, :].rearrange("p j (o d) -> p j o d", d=OD)
                eng.tensor_add(
                    y2v[:, :, :, 0:8], y2v[:, :, :, 0:8], y2v[:, :, :, 8:16])
                eng.tensor_add(
                    y2v[:, :, :, 0:4], y2v[:, :, :, 0:4], y2v[:, :, :, 4:8])
                eng.tensor_add(
                    y2v[:, :, :, 0:2], y2v[:, :, :, 0:2], y2v[:, :, :, 2:4])
                dst = blg[:, bblk, :, sl].transpose([0, 2, 1])
                if first:
                    eng.tensor_add(dst, y2v[:, :, :, 0], y2v[:, :, :, 1])
                else:
                    ts = tsp.tile([128, 20, OC], BF, tag="ts")
                    eng.tensor_add(ts[:, :n, :], y2v[:, :, :, 0], y2v[:, :, :, 1])
                    eng.tensor_add(dst, dst, ts[:, :n, :])

        # --- iter 0: c uniform -> s = 0.1 * sum_i u_hat (accumulated above),
        # one batched [32,160] squash, then per-bblk broadcast + agreement ---
        _squash(nc, smp, ps0[:], 0.1, v32)
        for bblk in range(NBB):
            agreement_bblk(bblk, first=True, vsrc=(v32, 32))

        # --- iters 1, 2: fully per-bblk pipelines so PE stage-1 of one
        # b-block overlaps the DVE agreement/softmax of another ---
        for it in (1, 2):
            for bblk in range(NBB):
                # softmax over o for this b-block (no max-sub: |logits| << 1)
                nc.scalar.activation(
                    c_sb[:, bblk, :, :], blg[:, bblk, :, :], AF.Exp)
                cf = c_sb[:, bblk, :, :].transpose([0, 2, 1])   # [p, i, o]
                sm = smp.tile([128, NIB], F32, tag="sm")
                nc.vector.tensor_reduce(
                    sm[:], cf, axis=AX.X, op=ALU.add)
                rr = smp.tile([128, NIB], F32, tag="rr")
                nc.vector.reciprocal(rr[:], sm[:])
                nc.vector.tensor_mul(
                    cf, cf, rr[:].unsqueeze(-1).broadcast_to((128, NIB, OC)))

                # stage 1+2: s = diag(C^T U) via blockdiag-c, o-mask, reduce
                cbt = (cb0, cb1)[bblk % 2]
                for bp in range(8):
                    nc.gpsimd.dma_start(
                        out=cbt[bp * 16:(bp + 1) * 16, bp * 10:(bp + 1) * 10, :],
                        in_=c_sb[bp * 16:(bp + 1) * 16, bblk, :, :])
                ps1 = pbig.tile([80, F], F32, tag="pbig", name=f"ps1_{bblk}")
                for j in range(NIB):
                    nc.tensor.matmul(
                        ps1[:], lhsT=cbt[:, :, j],
                        rhs=u_hat[:, j, bblk, :],
                        start=(j == 0), stop=(j == NIB - 1))
                mskd = mkp.tile([80, F], BF, tag="mskd")
                nc.vector.tensor_mul(mskd[:], ps1[:], msk_sb[:])
                psv = psml.tile([8, F], F32, tag="psml")
                nc.tensor.matmul(
                    psv[:], lhsT=o80_sb[:], rhs=mskd[:], start=True, stop=True)
                if it == 1:
                    _squash(nc, smp, psv, 1.0, v8[bblk])
                    agreement_bblk(bblk, first=False, vsrc=(v8[bblk], 8))
                else:
                    _squash(nc, smp, psv, 1.0, of8[bblk])
                    nc.scalar.dma_start(
                        out=out_d[bblk * 8:(bblk + 1) * 8, :], in_=of8[bblk][:])

    if split_waits:
        _split_multiwait(nc)
    return nc


def xw_view(ws_d, e0, e1):
    """[128(q), e0:e1, 9, F] view of ws_d [8, 128, 9, F] for the SBUF dst."""
    return ws_d[e0:e1].transpose([1, 0, 2, 3])


def _host_inputs(x, W):
    """Per-core input maps from full x [256,1152,8] f32, W [1,1152,10,16,8] f32."""
    bf = np.float16
    W0 = np.asarray(W[0], dtype=np.float32)
    # ws[e, q=(ii,k), j, (o,d)] = W[(e*9+j)*16+ii, o, d, k]
    ws = np.ascontiguousarray(
        W0.reshape(8, 9, 16, OC, OD, ID).transpose(0, 2, 5, 1, 3, 4)
        .reshape(8, 128, 9, F)).astype(bf)
    msk = np.zeros((80, F), dtype=bf)
    for bpp in range(8):
        for o in range(OC):
            msk[bpp * 10 + o, o * OD:(o + 1) * OD] = 1.0
    o80 = np.zeros((80, 8), dtype=bf)
    for p in range(80):
        o80[p, p // 10] = 1.0
    # sel[q, b, p] = 1 iff q == 8*b + p//16  (broadcast v-row of b-block b's
    # bp to the 16 partitions (bp, ii); [0:8, 0, :] doubles as the v8 form)
    sel = np.zeros((32, NBB, 128), dtype=bf)
    for b in range(NBB):
        for p in range(128):
            sel[8 * b + p // 16, b, p] = 1.0

    in_maps = []
    for c in range(N_CORES):
        xc = np.asarray(x[c * B_LOC:(c + 1) * B_LOC], dtype=np.float32)
        # bd[e, q=(ii,k), j, bb, m=(bp,ii')] = x[bb*8+bp, (e*9+j)*16+ii, k] iff ii'==ii
        r = xc.reshape(NBB, 8, 8, 9, 16, ID)          # [bb, bp, e, j, ii, k]
        bd6 = np.zeros((8, 16, ID, 9, NBB, 8, 16), dtype=np.float32)
        for ii in range(16):
            # [bb, bp, e, j, k] -> [e, k, j, bb, bp]
            bd6[:, ii, :, :, :, :, ii] = r[:, :, :, :, ii, :].transpose(2, 4, 3, 0, 1)
        bd = np.ascontiguousarray(bd6.reshape(8, 128, 9, NBB, 128)).astype(bf)
        # xt[q=(ii,k), iblk, b] = x[b, iblk*16+ii, k]
        xt = np.ascontiguousarray(
            xc.reshape(B_LOC, NIB, 16, ID).transpose(2, 3, 1, 0)
            .reshape(128, NIB, B_LOC)).astype(bf)
        in_maps.append(
            {"bd": bd, "xt": xt, "ws": ws, "msk": msk, "o80": o80, "sel": sel})
    return in_maps


_NC_CACHE = {}


def kernel(x, W):
    from concourse.bass_utils import run_bass_kernel_spmd

    if "nc" not in _NC_CACHE:
        _NC_CACHE["nc"] = build_program()
    nc = _NC_CACHE["nc"]
    in_maps = _host_inputs(x, W)
    res = run_bass_kernel_spmd(nc, in_maps, core_ids=list(range(N_CORES)))
    out = np.concatenate([r["out"] for r in res.results], axis=0)
    return out.reshape(B_FULL, OC, OD).astype(np.float32)


if __name__ == "__main__":
    nc = build_program()
    print("program built ok")
